# revision 39
# baseline (speedup 1.0000x reference)
"""AUC (histogram_binning) Trainium2 kernel.

Problem: AUC of sigmoid(output) vs one-hot(target), 30 descending thresholds
k/30, trapezoid rule.  output [500000, 64] f32, target [500000] int32.

Math: with b = floor(30*sigmoid(x)) in [0,29],
  tp_asc[j]  = #{rows n: b[n, target[n]] >= j+1}               (j = 0..29)
  fp_asc[j]  = #{all elements: b >= j+1} - tp_asc[j]
then tpr/fpr + trapezoid scan (tiny, done on host in f32 mirroring the
reference).

Device algorithm (per core, data-parallel over rows, 8 cores):
  1. stream x in chunks: ACT Sigmoid then ACT Identity(scale=30, bias=-0.5)
     with int16 output = bin index b (the HW f32->int cast rounds to
     nearest-even, so 30s - 0.5 implements floor(30s); ties measure-zero)
  2. one-hot pack: u = b + 64*[channel == target[row]]: the host ships
     d = channel - target per element, so the one-hot is a single fused
     (d==0)*64 tensor_scalar (4x mode) plus an in-place add on b
     - big counts on u:  #[u >= k] = #[b >= k over false] + (#true)  since
       true elements have u >= 64; so count_ge[k] = big[k] + tcnt[k] - T
     - true-element bins recovered per row with a window-64 max (TT pairwise
       max tree + final reduce): gmax[row] = 64 + b[row, target[row]]
  3. 30 threshold passes on u split 23/7 between DVE (tensor_scalar is_ge
     with fused accum_out, 4x mode) and ACT (Sign activation with fused
     accum_out, count = (sign_sum + #elems)/2); count passes for each
     quarter of u are emitted as soon as streaming completes it, so they
     overlap remaining chunks. 30 tiny DVE passes on gmax give tcnt.
  4. per-partition accumulators DMA'd out; host combines cores/partitions
     (exact integer counts) and runs the 31-point trapezoid in f32.

Engine budget (cost model): DVE ~252us (pack + window max + 23 thresholds),
ACT ~239us (sigmoid + binify + 7 thresholds), overlapped to ~269us e2e;
DMA (25MB in) fully hidden. The 30-threshold exact count is intrinsically
ALU-bound on this ISA: no scatter/histogram primitive exists, PE only
contracts over partitions (can't apply per-element nonlinearity), so every
element meets every threshold through DVE/ACT at <= 4 elem/cycle/lane.

Padding: shards are padded on the host to 128*489 = 62592 rows (pad x = -30
=> b = 0, pad target = 0). A pad row has one fake "true" at channel 0 with
u = 64: contributes 0 to every tcnt (needs u >= 64+k) and exactly +1 to
every big[k], cancelled by T=489 per partition in the count_ge formula.
Host-side padding keeps every SBUF tile single-writer (walrus has a tiny
per-instruction sync-wait budget).
"""

import os

import numpy as np

# ---------------- problem constants (hardcoded; kernel.py is standalone) ---
N, C = 500_000, 64
STEP = 30
NCORES = 8
ROWS = N // NCORES          # 62500 rows per core
P = 128                     # partitions
RPP = 489                   # rows per partition (padded): 128*489 = 62592
ROWS_PAD = P * RPP          # host pads each shard to this many rows
W = RPP * C                 # 31296 elements per partition
NQ = 4                      # count-pass quarters
QW = W // NQ                # 7824
CH_R = 50                   # rows per chunk per partition
EPS = 1e-8

# threshold split between engines (tuned from cost model: DVE ~8.4us/thr
# at 4x, ACT ~27us/thr at 1x)
DVE_KS = list(range(1, 24))          # 23 thresholds on DVE
ACT_KS = list(range(24, 31))         # 7 thresholds on ACT
# output tile layout (f32 columns)
OUT_DVE = 0                           # 4 quarter-accums per DVE threshold
OUT_ACT = OUT_DVE + 4 * len(DVE_KS)   # 4 quarter sign-sums per ACT threshold
OUT_TC = 240                          # 30 tcnt columns
OUT_W = 288

_BUILT = None


def _chunks():
    out = []
    r0 = 0
    while r0 < RPP:
        rc = min(CH_R, RPP - r0)
        out.append((r0, rc))
        r0 += rc
    return out


def _emit(nc, tc, xa, ta, out_ap):
    import concourse.mybir as mybir

    f32 = mybir.dt.float32
    i16 = mybir.dt.int16
    i32 = mybir.dt.int32
    Alu = mybir.AluOpType
    Act = mybir.ActivationFunctionType
    # inputs are host-padded to ROWS_PAD = 128*489 rows; ta is the target
    # replicated across the 64 channels (int16), same shape as xa
    xa_main = xa.rearrange("(p r) c -> p r c", p=P)
    ta_main = ta.rearrange("(p r) c -> p r c", p=P)

    if True:
        with tc.tile_pool(name="pool", bufs=1) as pool:
            # ---------------- persistent tiles
            B = pool.tile([P, W], i16)            # bins then packed u
            gmax = pool.tile([P, RPP], i16)
            # per-engine accumulator tiles (sharing one tile across engines
            # piles sync-waits onto single instructions; walrus rejects >N)
            cntD = pool.tile([P, NQ * len(DVE_KS)], f32)
            sgnA = pool.tile([P, NQ * len(ACT_KS)], f32)
            tcnt = pool.tile([P, STEP], f32)

            # ACT bias constants (DVE-written so ACT instructions only ever
            # wait on the DVE semaphore): sign biases -(k-0.5) + binify -0.5
            bias_t = pool.tile([P, len(ACT_KS) + 1 + STEP], f32)
            for i, k in enumerate(ACT_KS):
                nc.vector.memset(bias_t[:, i:i + 1], -(k - 0.5))
            bias_half = bias_t[:, len(ACT_KS):len(ACT_KS) + 1]
            nc.vector.memset(bias_half, -0.5)


            # ---------------- streaming chunks with interleaved counting:
            # emit each quarter's count passes right after the chunk that
            # completes it so ACT sign passes overlap later chunks' streaming
            scrD = pool.tile([P, QW], i16)
            scrA = pool.tile([P, QW], i16)

            def emit_quarter_counts(q):
                for i, k in enumerate(ACT_KS):
                    nc.scalar.activation(
                        out=scrA, in_=B[:, q * QW:(q + 1) * QW],
                        func=Act.Sign, bias=bias_t[:, i:i + 1], scale=1.0,
                        accum_out=sgnA[:, NQ * i + q:NQ * i + q + 1])
                for i, k in enumerate(DVE_KS):
                    nc.vector.tensor_scalar(
                        out=scrD, in0=B[:, q * QW:(q + 1) * QW],
                        scalar1=float(k), scalar2=None,
                        op0=Alu.is_ge, op1=Alu.add,
                        accum_out=cntD[:, NQ * i + q:NQ * i + q + 1])

            next_q = 0
            with tc.tile_pool(name="stream", bufs=2) as spool:
                for (r0, rc) in _chunks():
                    wc = rc * C
                    c0 = r0 * C
                    xc = spool.tile([P, CH_R * C], f32, tag="xc")
                    nc.sync.dma_start(out=xc[:, 0:wc],
                                      in_=xa_main[:, r0:r0 + rc, :])
                    trc = spool.tile([P, CH_R * C], i16, tag="trc")
                    nc.sync.dma_start(out=trc[:, 0:wc],
                                      in_=ta_main[:, r0:r0 + rc, :])
                    sc = spool.tile([P, CH_R * C], f32, tag="sc")
                    nc.scalar.activation(out=sc[:, 0:wc], in_=xc[:, 0:wc],
                                         func=Act.Sigmoid)
                    # b = floor(30*s): HW int16 cast rounds to nearest even,
                    # so compute 30*s - 0.5 (ties are measure-zero)
                    nc.scalar.activation(out=B[:, c0:c0 + wc], in_=sc[:, 0:wc],
                                         func=Act.Identity, scale=30.0,
                                         bias=bias_half)
                    # one-hot pack u = b + 64*[c == target]: host ships
                    # d = c - t, so oh64 = 64*(d == 0) is one single-src 4x op
                    oh = spool.tile([P, CH_R * C], i16, tag="oh")
                    nc.vector.tensor_scalar(
                        out=oh[:, 0:wc], in0=trc[:, 0:wc], scalar1=0.0,
                        scalar2=64.0, op0=Alu.is_equal, op1=Alu.mult)
                    nc.vector.tensor_tensor(
                        out=B[:, c0:c0 + wc], in0=B[:, c0:c0 + wc],
                        in1=oh[:, 0:wc], op=Alu.add)
                    # per-row true-bin extraction: pairwise-max tree over the
                    # 64-wide window (TT-max at 2x), final 16-wide reduce at 1x
                    uv = B[:, c0:c0 + wc].rearrange("p (r c) -> p r c", c=C)
                    m1 = spool.tile([P, CH_R * 32], i16, tag="m1")
                    m1v = m1[:, 0:rc * 32].rearrange("p (r c) -> p r c", c=32)
                    nc.vector.tensor_tensor(out=m1v, in0=uv[:, :, 0:32],
                                            in1=uv[:, :, 32:64], op=Alu.max)
                    m2 = spool.tile([P, CH_R * 16], i16, tag="m2")
                    m2v = m2[:, 0:rc * 16].rearrange("p (r c) -> p r c", c=16)
                    nc.vector.tensor_tensor(out=m2v, in0=m1v[:, :, 0:16],
                                            in1=m1v[:, :, 16:32], op=Alu.max)
                    m3 = spool.tile([P, CH_R * 8], i16, tag="m3")
                    m3v = m3[:, 0:rc * 8].rearrange("p (r c) -> p r c", c=8)
                    nc.vector.tensor_tensor(out=m3v, in0=m2v[:, :, 0:8],
                                            in1=m2v[:, :, 8:16], op=Alu.max)
                    nc.vector.tensor_reduce(
                        out=gmax[:, r0:r0 + rc], in_=m3v,
                        axis=mybir.AxisListType.X, op=Alu.max)
                    while next_q < NQ and (r0 + rc) * C >= (next_q + 1) * QW:
                        emit_quarter_counts(next_q)
                        next_q += 1

            # ---------------- remaining counting passes
            while next_q < NQ:
                emit_quarter_counts(next_q)
                next_q += 1
            scrT = pool.tile([P, RPP], i16)
            for j, k in enumerate(range(1, 31)):
                nc.vector.tensor_scalar(
                    out=scrT, in0=gmax, scalar1=float(64 + k), scalar2=None,
                    op0=Alu.is_ge, op1=Alu.add,
                    accum_out=tcnt[:, j:j + 1])

            nc.sync.dma_start(out=out_ap[:, OUT_DVE:OUT_DVE + NQ * len(DVE_KS)],
                              in_=cntD)
            nc.sync.dma_start(out=out_ap[:, OUT_ACT:OUT_ACT + NQ * len(ACT_KS)],
                              in_=sgnA)
            nc.sync.dma_start(out=out_ap[:, OUT_TC:OUT_TC + STEP], in_=tcnt)


def _build():
    import concourse.bacc as bacc
    import concourse.mybir as mybir
    from concourse import tile

    nc = bacc.Bacc("TRN2", target_bir_lowering=False, debug=False)
    x_d = nc.dram_tensor("x", [ROWS_PAD, C], mybir.dt.float32, kind="ExternalInput")
    t_d = nc.dram_tensor("tr", [ROWS_PAD, C], mybir.dt.int16, kind="ExternalInput")
    out_d = nc.dram_tensor("out", [P, OUT_W], mybir.dt.float32, kind="ExternalOutput")
    with tile.TileContext(nc) as tc:
        _emit(nc, tc, x_d.ap(), t_d.ap(), out_d.ap())
    nc.compile()   # bacc passes: wait splitting, reg alloc, DCE, nop fusion
    return nc


def _get_nc():
    global _BUILT
    if _BUILT is None:
        _BUILT = _build()
    return _BUILT


def _combine(results):
    """results: list of per-core {"out": [P, OUT_W] f32} -> AUC scalar f32."""
    big = np.zeros(STEP + 1, np.float64)   # big[k] for k=1..30 at index k-1
    tcnt = np.zeros(STEP, np.float64)
    for r in results:
        o = r["out"].astype(np.float64)
        for i, k in enumerate(DVE_KS):
            big[k - 1] += o[:, OUT_DVE + NQ * i:OUT_DVE + NQ * (i + 1)].sum()
        for i, k in enumerate(ACT_KS):
            sgn = o[:, OUT_ACT + NQ * i:OUT_ACT + NQ * (i + 1)].sum()
            big[k - 1] += (sgn + P * W) / 2.0
        tcnt += o[:, OUT_TC:OUT_TC + STEP].sum(axis=0)
    big = big[:STEP]
    # count_ge[k] = big[k] + tcnt[k] - T  (T = 489 fake/true rows per partition)
    total_T = float(NCORES * P * RPP)
    count_ge = big + tcnt - total_T

    tp_asc = tcnt.astype(np.float32)                       # [30], k=1..30
    fp_asc = (count_ge - tcnt).astype(np.float32)
    trues = np.float32(N)
    falses = np.float32(np.float32(N * C) - trues)
    tpr = (tp_asc / (trues + np.float32(EPS))).astype(np.float32)
    fpr = (fp_asc / (falses + np.float32(EPS))).astype(np.float32)
    tpr = tpr[::-1]   # descending-threshold order
    fpr = fpr[::-1]
    tprs = np.concatenate([np.zeros(1, np.float32), tpr])
    fprs = np.concatenate([np.zeros(1, np.float32), fpr])
    width = np.abs(fprs[1:] - fprs[:-1])
    tmin = np.minimum(tprs[1:], tprs[:-1])
    tmax = np.maximum(tprs[1:], tprs[:-1])
    area = np.sum(width * tmin + np.float32(0.5) * width * (tmax - tmin),
                  dtype=np.float32)
    return np.float32(area)


LAST_RESULT = None


def _pad_x(x):
    out = np.full((ROWS_PAD, C), -30.0, dtype=np.float32)
    out[:ROWS] = x
    return out


def _pad_t(t):
    tp = np.zeros(ROWS_PAD, dtype=np.int16)
    tp[:ROWS] = t.astype(np.int16)
    # d[n, c] = c - t_n: the device one-hot is just (d == 0), no iota needed
    return np.ascontiguousarray(
        np.arange(C, dtype=np.int16)[None, :] - tp[:, None])


def kernel(output, target):
    # NTFF tracing needs axon hook modules that aren't shipped in this
    # container; make sure run_bass_kernel_spmd never tries.
    os.environ["BASS_NEVER_TRACE"] = "1"
    from concourse import bass_utils

    global LAST_RESULT
    nc = _get_nc()
    in_maps = []
    for i in range(NCORES):
        sl = slice(i * ROWS, (i + 1) * ROWS)
        in_maps.append({"x": _pad_x(output[sl]), "tr": _pad_t(target[sl])})
    res = bass_utils.run_bass_kernel_spmd(nc, in_maps, core_ids=list(range(NCORES)))
    LAST_RESULT = res
    return np.asarray(_combine(res.results), dtype=np.float32)
